# revision 20
# baseline (speedup 1.0000x reference)
"""2-layer GCN (GCNConv x2 + ReLU) on 8 Trainium2 NeuronCores.

Math (symmetric-norm GCN with self-loops; W commutes past aggregation):
    deg[d]  = in_degree(d) + 1,  dinv = deg^-1/2
    agg1[d] = sum_{(s,d) in E} dinv[s]*x[s] + dinv[d]*x[d]
    h       = relu((dinv[d] * agg1[d]) @ W1 + b1)
    hs2     = h * dinv[:,None]                        (bf16, cols padded to 128)
    agg2[d] = sum_{(s,d) in E} hs2[s] + hs2[d]
    out     = (dinv[d] * agg2[d]) @ W2 + b2

Distribution: edge-parallel. dst nodes (and their in-edges) are sharded over
8 cores. Layer 1 inputs are distributed in edge-expanded form (each core
receives the raw bf16 x rows of its edges' sources, in its aggregation
schedule order) and streamed sequentially at full DMA bandwidth; the
dinv[src] normalization is applied on device. Layer 2's table (h*dinv,
device-computed) is all-gathered and fetched with dma_gather (random 256B
bf16 rows). Aggregation = one-hot bf16 selector matmuls accumulating
per-dst-tile sums in PSUM fp32; self-loop terms are injected via identity
matmuls that also zero the PSUM banks (start=True).

Host-side work is index/layout preprocessing and index-driven data movement
only (edge sort/partition, degree counts, padding, int16 index packing,
dtype conversion, edge-expansion of the input shard); all FLOPs on x/W/b
run on device.
"""

import math
import sys

import numpy as np
import ml_dtypes

sys.path.insert(0, "/opt/trn_rl_repo")

import concourse.bacc as bacc
import concourse.bass as bass
import concourse.mybir as mybir
import concourse.tile as tile

FP32 = mybir.dt.float32
BF16 = mybir.dt.bfloat16
I16 = mybir.dt.int16
AL = mybir.AluOpType
AF = mybir.ActivationFunctionType


class Cfg:
    def __init__(self, n_nodes, n_edges, f_in=128, hid=64, f_out=128,
                 ncores=8, tiles_per_core=None, group1=8, group2=8, nchunk=4,
                 subb=96):
        self.N = n_nodes
        self.E = n_edges
        self.F_IN = f_in
        self.HID = hid
        self.F_OUT = f_out
        self.NC = ncores
        if tiles_per_core is None:
            tiles_per_core = math.ceil(n_nodes / (ncores * 128))
        self.TPC = tiles_per_core
        self.NN = tiles_per_core * 128          # nodes per core (padded)
        self.NP = self.NN * ncores              # padded node count
        self.NTILE = self.NP // 128
        assert self.NP >= n_nodes
        self.NCHUNK = nchunk
        assert self.NP % nchunk == 0
        self.CHUNK = self.NP // nchunk          # gather-table rows per chunk
        assert self.CHUNK <= 32767, "int16 gather index range"
        self.G1 = group1                        # layer-1 dst tiles per group
        self.G2 = group2                        # layer-2 dst tiles per group
        self.SUBB = subb                        # layer-1 stream blocks/subcall


REAL_CFG = Cfg(100000, 3200000)


# ----------------------------------------------------------------------------
# Host preprocessing
# ----------------------------------------------------------------------------

def _sched2(cfg, src, dst):
    """Layer-2 gather schedule: (group2, chunk) calls, per-(tile,chunk) segs
    padded to 128, SPMD-uniform across cores. Returns calls, idx, dloc."""
    NP, NN, TPC, NC = cfg.NP, cfg.NN, cfg.TPC, cfg.NC
    G, NCHUNK, CHUNK = cfg.G2, cfg.NCHUNK, cfg.CHUNK
    NGRP = math.ceil(TPC / G)

    core_of = dst // NN
    tile_of = (dst % NN) // 128
    dloc_of = dst % 128
    chunk_of = src // CHUNK

    key = (core_of * TPC + tile_of) * NCHUNK + chunk_of
    counts = np.bincount(key, minlength=NC * TPC * NCHUNK).reshape(
        NC, TPC, NCHUNK)
    seg_len = counts.max(axis=0)
    seg_len = (np.ceil(seg_len / 128).astype(np.int64)) * 128

    order = np.argsort(key, kind="stable")
    src_s, key_s = src[order], key[order]
    dloc_s = dloc_of[order]
    run_starts = np.searchsorted(key_s, np.arange(NC * TPC * NCHUNK))

    TOT = int(seg_len.sum())
    NBLK = TOT // 128
    idx_all = np.zeros((NC, 128, TOT // 16), dtype=np.int16)
    dloc_all = np.full((NC, 128, NBLK), 300.0, dtype=np.float32)

    calls = []
    pos = 0
    for g in range(NGRP):
        t0, t1 = g * G, min((g + 1) * G, TPC)
        for c in range(NCHUNK):
            Lgc = int(seg_len[t0:t1, c].sum())
            if Lgc == 0:
                continue
            blocks = []
            for t in range(t0, t1):
                blocks += [t] * int(seg_len[t, c] // 128)
            SUB = 96
            for sb0 in range(0, len(blocks), SUB):
                sb = blocks[sb0:sb0 + SUB]
                calls.append({"g": g, "c": c, "L": len(sb) * 128,
                              "col16": pos // 16 + sb0 * 8,
                              "blk0": pos // 128 + sb0, "blocks": sb})
            for core in range(NC):
                p = pos
                for t in range(t0, t1):
                    L = int(seg_len[t, c])
                    if L == 0:
                        continue
                    k = (core * TPC + t) * NCHUNK + c
                    s0 = run_starts[k]
                    n = int(counts[core, t, c])
                    seg_idx = np.zeros(L, dtype=np.int16)
                    seg_dl = np.full(L, 300.0, dtype=np.float32)
                    if n:
                        seg_idx[:n] = (src_s[s0:s0 + n] - c * CHUNK).astype(
                            np.int16)
                        seg_dl[:n] = dloc_s[s0:s0 + n]
                    w = seg_idx.reshape(L // 16, 16).T
                    idx_all[core][:, p // 16: p // 16 + L // 16] = \
                        np.tile(w, (8, 1))
                    dloc_all[core][:, p // 128: p // 128 + L // 128] = \
                        seg_dl.reshape(L // 128, 128).T
                    p += L
            pos += Lgc
    assert pos == TOT
    return {"calls": calls, "TOT": TOT, "NBLK": NBLK,
            "maxB": max(c["L"] for c in calls) // 128,
            "idx": idx_all, "dloc": dloc_all}


def _sched1(cfg, src, dst, dinv):
    """Layer-1 stream schedule: chunkless, per-tile segs padded to 128,
    split into subcalls of <= SUBB blocks within each group1.
    Returns calls, per-core (srcs, dloc, dsrc)."""
    NN, TPC, NC, G = cfg.NN, cfg.TPC, cfg.NC, cfg.G1
    NGRP = math.ceil(TPC / G)

    core_of = dst // NN
    tile_of = (dst % NN) // 128
    dloc_of = dst % 128

    key = core_of * TPC + tile_of
    counts = np.bincount(key, minlength=NC * TPC).reshape(NC, TPC)
    seg_len = counts.max(axis=0)
    seg_len = (np.ceil(seg_len / 128).astype(np.int64)) * 128

    order = np.argsort(key, kind="stable")
    src_s, key_s = src[order], key[order]
    dloc_s = dloc_of[order]
    run_starts = np.searchsorted(key_s, np.arange(NC * TPC))

    TOT = int(seg_len.sum())
    NBLK = TOT // 128
    srcs_all = np.zeros((NC, TOT), dtype=np.int64)
    dloc_all = np.full((NC, 128, NBLK), 300.0, dtype=np.float32)
    dsrc_all = np.zeros((NC, 128, NBLK), dtype=np.float32)

    calls = []
    pos = 0
    for g in range(NGRP):
        t0, t1 = g * G, min((g + 1) * G, TPC)
        blocks = []
        for t in range(t0, t1):
            blocks += [t] * int(seg_len[t] // 128)
        for sb0 in range(0, len(blocks), cfg.SUBB):
            sb = blocks[sb0:sb0 + cfg.SUBB]
            calls.append({"g": g, "L": len(sb) * 128,
                          "blk0": pos // 128 + sb0, "blocks": sb})
        for core in range(NC):
            p = pos * 1
            for t in range(t0, t1):
                L = int(seg_len[t])
                k = core * TPC + t
                sidx = run_starts[k]
                n = int(counts[core, t])
                seg_src = np.zeros(L, dtype=np.int64)
                seg_dl = np.full(L, 300.0, dtype=np.float32)
                seg_ds = np.zeros(L, dtype=np.float32)
                if n:
                    seg_src[:n] = src_s[sidx:sidx + n]
                    seg_dl[:n] = dloc_s[sidx:sidx + n]
                    seg_ds[:n] = dinv[seg_src[:n]]
                srcs_all[core][p:p + L] = seg_src
                dloc_all[core][:, p // 128: p // 128 + L // 128] = \
                    seg_dl.reshape(L // 128, 128).T
                dsrc_all[core][:, p // 128: p // 128 + L // 128] = \
                    seg_ds.reshape(L // 128, 128).T
                p += L
        pos += int(seg_len[t0:t1].sum())
    assert pos == TOT
    return {"calls": calls, "TOT": TOT, "NBLK": NBLK,
            "srcs": srcs_all, "dloc": dloc_all, "dsrc": dsrc_all}


def preprocess(cfg, x, edge_index, W1, b1, W2, b2):
    N, NP, NN, TPC, NC = cfg.N, cfg.NP, cfg.NN, cfg.TPC, cfg.NC

    src = np.asarray(edge_index[0], dtype=np.int64)
    dst = np.asarray(edge_index[1], dtype=np.int64)

    deg = np.bincount(dst, minlength=NP).astype(np.float32) + 1.0
    dinv = (1.0 / np.sqrt(deg)).astype(np.float32)

    s1 = _sched1(cfg, src, dst, dinv)
    s2 = _sched2(cfg, src, dst)
    meta = {"s1": s1, "s2": s2}

    xb = np.zeros((NP, cfg.F_IN), dtype=ml_dtypes.bfloat16)
    xb[:N] = np.asarray(x, dtype=np.float32).astype(ml_dtypes.bfloat16)
    dinv_full = np.ascontiguousarray(dinv.reshape(cfg.NTILE, 128).T)
    GW = max(cfg.G1, cfg.G2)
    iota = np.tile(np.tile(np.arange(128, dtype=np.float32), GW)[None, :],
                   (128, 1)).astype(ml_dtypes.bfloat16)
    ident = np.eye(128, dtype=np.float32).astype(ml_dtypes.bfloat16)

    in_maps = []
    for core in range(NC):
        sh = slice(core * NN, (core + 1) * NN)
        # edge-expanded layer-1 stream, slot-major [128, NBLK1*128]
        mx = xb[s1["srcs"][core]]                       # [TOT1, 128] bf16
        mx = np.ascontiguousarray(
            mx.reshape(s1["NBLK"], 128, cfg.F_IN).transpose(1, 0, 2)
            .reshape(128, s1["NBLK"] * cfg.F_IN))
        in_maps.append({
            "mx": mx,
            "dloc1": s1["dloc"][core].astype(ml_dtypes.bfloat16),
            "dsrc1": s1["dsrc"][core].astype(ml_dtypes.bfloat16),
            "idx2": s2["idx"][core],
            "dloc2": s2["dloc"][core].astype(ml_dtypes.bfloat16),
            "xb_own": np.ascontiguousarray(xb[sh]),
            "dinv_own": np.ascontiguousarray(
                dinv_full[:, core * TPC:(core + 1) * TPC]),
            "W1": np.asarray(W1, np.float32).astype(ml_dtypes.bfloat16),
            "W2": np.asarray(W2, np.float32).astype(ml_dtypes.bfloat16),
            "b1": np.tile(np.asarray(b1, np.float32)[None, :], (128, 1)),
            "b2": np.tile(np.asarray(b2, np.float32)[None, :], (128, 1)),
            "iota": iota,
            "ident": ident,
        })
    return in_maps, meta, dinv


# ----------------------------------------------------------------------------
# Device graph
# ----------------------------------------------------------------------------

def build_bass(cfg, meta, debug=False):
    NN, TPC, HID, F_IN, F_OUT = cfg.NN, cfg.TPC, cfg.HID, cfg.F_IN, cfg.F_OUT
    NP, NTILE, NC, CHUNK = cfg.NP, cfg.NTILE, cfg.NC, cfg.CHUNK
    G1, G2 = cfg.G1, cfg.G2
    s1, s2 = meta["s1"], meta["s2"]
    NGRP1 = math.ceil(TPC / G1)
    NGRP2 = math.ceil(TPC / G2)
    MAXB = max(cfg.SUBB, s2["maxB"])
    assert MAXB <= 96, MAXB

    nc = bacc.Bacc("TRN2", target_bir_lowering=False, debug=debug)

    mxp = nc.declare_dram_parameter("mx", [128, s1["NBLK"] * 128], BF16,
                                    isOutput=False)
    dloc1p = nc.declare_dram_parameter("dloc1", [128, s1["NBLK"]], BF16,
                                       isOutput=False)
    dsrc1p = nc.declare_dram_parameter("dsrc1", [128, s1["NBLK"]], BF16,
                                       isOutput=False)
    idx2p = nc.declare_dram_parameter("idx2", [128, s2["TOT"] // 16], I16,
                                      isOutput=False)
    dloc2p = nc.declare_dram_parameter("dloc2", [128, s2["NBLK"]], BF16,
                                       isOutput=False)
    xbop = nc.declare_dram_parameter("xb_own", [NN, F_IN], BF16,
                                     isOutput=False)
    dinvop = nc.declare_dram_parameter("dinv_own", [128, TPC], FP32,
                                       isOutput=False)
    W1p = nc.declare_dram_parameter("W1", [F_IN, HID], BF16, isOutput=False)
    W2p = nc.declare_dram_parameter("W2", [HID, F_OUT], BF16, isOutput=False)
    b1p = nc.declare_dram_parameter("b1", [128, HID], FP32, isOutput=False)
    b2p = nc.declare_dram_parameter("b2", [128, F_OUT], FP32, isOutput=False)
    GW = max(G1, G2)
    iotap = nc.declare_dram_parameter("iota", [128, GW * 128], BF16,
                                      isOutput=False)
    identp = nc.declare_dram_parameter("ident", [128, 128], BF16,
                                       isOutput=False)
    out = nc.declare_dram_parameter("out", [NN, F_OUT], FP32, isOutput=True)

    groups = [list(range(NC))]

    with tile.TileContext(nc) as tc:
        with (
            tc.tile_pool(name="persist", bufs=1) as pp,
            tc.tile_pool(name="dram", bufs=1, space="DRAM") as dp,
            tc.tile_pool(name="xs", bufs=3) as xpool,
            tc.tile_pool(name="idxp", bufs=8) as ipool,
            tc.tile_pool(name="gatp", bufs=3) as gpool,
            tc.tile_pool(name="sel", bufs=8) as spool,
            tc.tile_pool(name="ps_acc", bufs=2, space="PSUM") as ps_acc,
            tc.tile_pool(name="ps_e", bufs=4, space="PSUM") as ps_e,
            tc.tile_pool(name="epi", bufs=4) as epool,
        ):
            # ---- persistent SBUF ----
            W1_s = pp.tile([F_IN, HID], BF16)
            W2_s = pp.tile([HID, F_OUT], BF16)
            b1_s = pp.tile([128, HID], FP32)
            b2_s = pp.tile([128, F_OUT], FP32)
            dinvo_s = pp.tile([128, TPC], FP32)
            iota_s = pp.tile([128, GW * 128], BF16)
            ident_s = pp.tile([128, 128], BF16)
            dloc1_s = pp.tile([128, s1["NBLK"]], BF16)
            dsrc1_s = pp.tile([128, s1["NBLK"]], BF16)
            dloc2_s = pp.tile([128, s2["NBLK"]], BF16)
            xd_own = pp.tile([128, TPC * 128], BF16)
            hs2 = pp.tile([128, TPC * 128], BF16)
            nc.vector.memset(hs2[:], 0.0)
            for t_, d_ in ((W1_s, W1p), (W2_s, W2p), (b1_s, b1p), (b2_s, b2p),
                           (dinvo_s, dinvop), (iota_s, iotap),
                           (ident_s, identp), (dloc1_s, dloc1p),
                           (dsrc1_s, dsrc1p), (dloc2_s, dloc2p)):
                nc.sync.dma_start(out=t_[:], in_=d_[:, :])

            # ---- DRAM temps ----
            shard2 = dp.tile([NN, 128], BF16)
            table2 = dp.tile([NP, 128], BF16)

            # ---- own-shard x*dinv tiles in SBUF (self-loop terms) ----
            XB = 8
            for b0 in range(0, TPC, XB):
                nb_ = min(XB, TPC - b0)
                xt = xpool.tile([128, XB * 128], BF16, tag="xt")
                nc.sync.dma_start(
                    out=xt[:, :nb_ * 128].rearrange("p (t f) -> p t f", f=F_IN),
                    in_=xbop[b0 * 128:(b0 + nb_) * 128, :].rearrange(
                        "(t p) f -> p t f", p=128))
                nc.vector.tensor_tensor(
                    out=xd_own[:, b0 * 128:(b0 + nb_) * 128].rearrange(
                        "p (t f) -> p t f", f=F_IN),
                    in0=xt[:, :nb_ * 128].rearrange("p (t f) -> p t f", f=F_IN),
                    in1=dinvo_s[:, b0:b0 + nb_].to_broadcast([128, nb_, F_IN]),
                    op=AL.mult)

            def self_matmuls(pacc, t0, TG, F, own_sb):
                for tt in range(TG):
                    col0 = tt * F
                    first = (col0 % 512) == 0
                    nc.tensor.matmul(
                        out=pacc[:, col0:col0 + F],
                        lhsT=ident_s[:],
                        rhs=own_sb[:, (t0 + tt) * 128:(t0 + tt) * 128 + F],
                        start=first, stop=first,
                        skip_group_check=True)

            def sel_build(dloc_sb, blk0, j, w):
                selw = spool.tile([128, GW * 128], BF16, tag="selw",
                                  name="selw")
                nc.vector.tensor_tensor(
                    out=selw[:, :w * 128].rearrange("p (b m) -> p b m", m=128),
                    in0=iota_s[:, :w * 128].rearrange("p (b m) -> p b m",
                                                      m=128),
                    in1=dloc_sb[:, blk0 + j:blk0 + j + w
                                ].to_broadcast([128, w, 128]),
                    op=AL.is_equal)
                return selw

            # ================= layer 1: streamed aggregation =================
            def layer1():
                G = G1
                for g in range(NGRP1):
                    t0 = g * G
                    TG = min(G, TPC - t0)
                    pacc = ps_acc.tile([128, G1 * 128], FP32, space="PSUM",
                                       tag="pacc", name="pacc")
                    self_matmuls(pacc, t0, TG, 128, xd_own)
                    for call in s1["calls"]:
                        if call["g"] != g:
                            continue
                        nb, blk0 = call["L"] // 128, call["blk0"]
                        gat = gpool.tile([128, MAXB * 128], BF16, tag="gat",
                                         name="gat")
                        nc.sync.dma_start(
                            out=gat[:, :nb * 128],
                            in_=mxp[:, blk0 * 128:(blk0 + nb) * 128])
                        # scale by dinv[src]
                        nc.vector.tensor_tensor(
                            out=gat[:, :nb * 128].rearrange(
                                "p (b f) -> p b f", f=128),
                            in0=gat[:, :nb * 128].rearrange(
                                "p (b f) -> p b f", f=128),
                            in1=dsrc1_s[:, blk0:blk0 + nb
                                        ].to_broadcast([128, nb, 128]),
                            op=AL.mult)
                        selw = None
                        for j, t in enumerate(call["blocks"]):
                            if j % G == 0:
                                w = min(G, nb - j)
                                selw = sel_build(dloc1_s, blk0, j, w)
                            nc.tensor.matmul(
                                out=pacc[:, (t - t0) * 128:(t - t0 + 1) * 128],
                                lhsT=selw[:, (j % G) * 128:(j % G + 1) * 128],
                                rhs=gat[:, j * 128:(j + 1) * 128],
                                start=False, stop=False,
                                skip_group_check=True)
                    for tt in range(TG):
                        t = t0 + tt
                        v = epool.tile([128, 128], BF16, tag="v", name="v")
                        nc.scalar.activation(
                            out=v[:], in_=pacc[:, tt * 128:(tt + 1) * 128],
                            func=AF.Copy, scale=dinvo_s[:, t:t + 1])
                        pt = ps_e.tile([128, 128], BF16, space="PSUM",
                                       tag="t", bufs=2, name="pt")
                        nc.tensor.transpose(out=pt[:], in_=v[:],
                                            identity=ident_s[:])
                        vT = epool.tile([128, 128], BF16, tag="vT", name="vT")
                        nc.scalar.activation(out=vT[:], in_=pt[:],
                                             func=AF.Copy)
                        ph = ps_e.tile([128, 128], FP32, space="PSUM", tag="e",
                                       bufs=2, name="ph")
                        nc.tensor.matmul(out=ph[:, :HID], lhsT=vT[:],
                                         rhs=W1_s[:], start=True, stop=True)
                        hh = epool.tile([128, HID], FP32, tag="hh", name="hh")
                        nc.vector.tensor_tensor(out=hh[:], in0=ph[:, :HID],
                                                in1=b1_s[:], op=AL.add)
                        nc.scalar.activation(
                            out=hs2[:, t * 128:t * 128 + HID], in_=hh[:],
                            func=AF.Relu, scale=dinvo_s[:, t:t + 1])

            # ================= layer 2: gathered aggregation =================
            def layer2():
                G = G2
                for g in range(NGRP2):
                    t0 = g * G
                    TG = min(G, TPC - t0)
                    pacc = ps_acc.tile([128, G2 * HID], FP32, space="PSUM",
                                       tag="pacc", name="pacc")
                    self_matmuls(pacc, t0, TG, HID, hs2)
                    for call in s2["calls"]:
                        if call["g"] != g:
                            continue
                        c, L = call["c"], call["L"]
                        nb, blk0 = L // 128, call["blk0"]
                        idxt = ipool.tile([128, (MAXB * 128) // 16], I16,
                                          tag="idx", name="idxt")
                        nc.sync.dma_start(
                            out=idxt[:, :L // 16],
                            in_=idx2p[:, call["col16"]:call["col16"] + L // 16])
                        gat = gpool.tile([128, MAXB * 128], BF16, tag="gat",
                                         name="gat")
                        nc.gpsimd.dma_gather(
                            out_ap=gat[:, :nb * 128].rearrange(
                                "p (b f) -> p b f", f=128),
                            in_ap=table2[c * CHUNK:(c + 1) * CHUNK, :],
                            idxs_ap=idxt[:, :L // 16],
                            num_idxs=L, num_idxs_reg=L, elem_size=128,
                            single_packet=False)
                        selw = None
                        for j, t in enumerate(call["blocks"]):
                            if j % G == 0:
                                w = min(G, nb - j)
                                selw = sel_build(dloc2_s, blk0, j, w)
                            nc.tensor.matmul(
                                out=pacc[:, (t - t0) * HID:(t - t0 + 1) * HID],
                                lhsT=selw[:, (j % G) * 128:(j % G + 1) * 128],
                                rhs=gat[:, j * 128:j * 128 + HID],
                                start=False, stop=False,
                                skip_group_check=True)
                    for tt in range(TG):
                        t = t0 + tt
                        u2 = epool.tile([128, HID], BF16, tag="v", name="u2")
                        nc.scalar.activation(
                            out=u2[:], in_=pacc[:, tt * HID:(tt + 1) * HID],
                            func=AF.Copy, scale=dinvo_s[:, t:t + 1])
                        pt = ps_e.tile([128, 128], BF16, space="PSUM",
                                       tag="t", bufs=2, name="pt")
                        nc.tensor.transpose(out=pt[:HID, :], in_=u2[:],
                                            identity=ident_s[:])
                        u2T = epool.tile([HID, 128], BF16, tag="vT", name="u2T")
                        nc.scalar.activation(out=u2T[:], in_=pt[:HID, :],
                                             func=AF.Copy)
                        po = ps_e.tile([128, 128], FP32, space="PSUM", tag="e",
                                       bufs=2, name="po")
                        nc.tensor.matmul(out=po[:, :F_OUT], lhsT=u2T[:],
                                         rhs=W2_s[:], start=True, stop=True)
                        oo = epool.tile([128, F_OUT], FP32, tag="oo", name="oo")
                        nc.vector.tensor_tensor(out=oo[:], in0=po[:, :F_OUT],
                                                in1=b2_s[:], op=AL.add)
                        nc.sync.dma_start(out=out[t * 128:(t + 1) * 128, :],
                                          in_=oo[:])

            layer1()
            nc.sync.dma_start(
                out=shard2[:].rearrange("(t p) f -> p t f", p=128),
                in_=hs2[:].rearrange("p (t f) -> p t f", f=128))
            nc.gpsimd.collective_compute(
                "AllGather", AL.bypass, replica_groups=groups,
                ins=[shard2[:].opt()], outs=[table2[:].opt()])
            layer2()

    return nc


def hoist_gather_waits(nc):
    """walrus's ANT codegen dies ("Reg has not been allocated yet") when a
    DMAGatherAnt carries an attached semaphore wait. Move any waits Tile
    attached onto a fresh no-op right before the gather."""
    gather_ops = (mybir.InstDMAGatherAnt, mybir.InstDMAScatterAddAnt)
    for blk in nc.main_func.blocks:
        insts = blk.instructions
        i = 0
        while i < len(insts):
            ins = insts[i]
            if isinstance(ins, gather_ops) and ins.sync_info is not None \
                    and len(ins.sync_info.on_wait) > 0:
                nop = mybir.InstNoOp(
                    name=f"gw-nop-{ins.name}",
                    ins=[], outs=[],
                    engine=ins.engine,
                    sync_info=mybir.SyncInfo(
                        on_wait=list(ins.sync_info.on_wait), on_update=[]),
                    text_hint="hoisted-gather-waits",
                    bass_nofuse=True,
                )
                ins.sync_info.on_wait = []
                insts.insert(i, nop)
                i += 1
            i += 1


# ----------------------------------------------------------------------------
# Entry points
# ----------------------------------------------------------------------------

def run_on_hw(cfg, in_maps, meta, trace=False, tmpdir=None):
    from concourse.bass_utils import run_bass_kernel_spmd
    nc = build_bass(cfg, meta, debug=False)
    hoist_gather_waits(nc)
    nc.finalize()
    res = run_bass_kernel_spmd(nc, in_maps, core_ids=list(range(cfg.NC)),
                               trace=trace, tmpdir=tmpdir)
    outs = [res.results[c]["out"] for c in range(cfg.NC)]
    full = np.concatenate(outs, axis=0)[:cfg.N]
    return full, res


def kernel(x, edge_index, W1, b1, W2, b2):
    cfg = REAL_CFG
    in_maps, meta, _ = preprocess(cfg, x, edge_index, W1, b1, W2, b2)
    out, _ = run_on_hw(cfg, in_maps, meta, trace=False)
    return out.astype(np.float32)


# revision 21
# speedup vs baseline: 1.0431x; 1.0431x over previous
"""2-layer GCN (GCNConv x2 + ReLU) on 8 Trainium2 NeuronCores.

Math (symmetric-norm GCN with self-loops; W commutes past aggregation):
    deg[d]  = in_degree(d) + 1,  dinv = deg^-1/2
    agg1[d] = sum_{(s,d) in E} dinv[s]*x[s] + dinv[d]*x[d]
    h       = relu((dinv[d] * agg1[d]) @ W1 + b1)
    hs2     = h * dinv[:,None]                        (bf16, cols padded to 128)
    agg2[d] = sum_{(s,d) in E} hs2[s] + hs2[d]
    out     = (dinv[d] * agg2[d]) @ W2 + b2

Distribution: edge-parallel. dst nodes (and their in-edges) are sharded over
8 cores. Layer 1 inputs are distributed in edge-expanded form (each core
receives the raw bf16 x rows of its edges' sources, in its aggregation
schedule order) and streamed sequentially at full DMA bandwidth; the
dinv[src] normalization is applied on device. Layer 2's table (h*dinv,
device-computed) is all-gathered and fetched with dma_gather (random 256B
bf16 rows). Aggregation = one-hot bf16 selector matmuls accumulating
per-dst-tile sums in PSUM fp32; self-loop terms are injected via identity
matmuls that also zero the PSUM banks (start=True).

Host-side work is index/layout preprocessing and index-driven data movement
only (edge sort/partition, degree counts, padding, int16 index packing,
dtype conversion, edge-expansion of the input shard); all FLOPs on x/W/b
run on device.
"""

import math
import sys

import numpy as np
import ml_dtypes

sys.path.insert(0, "/opt/trn_rl_repo")

import concourse.bacc as bacc
import concourse.bass as bass
import concourse.mybir as mybir
import concourse.tile as tile

FP32 = mybir.dt.float32
BF16 = mybir.dt.bfloat16
I16 = mybir.dt.int16
AL = mybir.AluOpType
AF = mybir.ActivationFunctionType


class Cfg:
    def __init__(self, n_nodes, n_edges, f_in=128, hid=64, f_out=128,
                 ncores=8, tiles_per_core=None, group1=8, group2=8, nchunk=4,
                 subb=96):
        self.N = n_nodes
        self.E = n_edges
        self.F_IN = f_in
        self.HID = hid
        self.F_OUT = f_out
        self.NC = ncores
        if tiles_per_core is None:
            tiles_per_core = math.ceil(n_nodes / (ncores * 128))
        self.TPC = tiles_per_core
        self.NN = tiles_per_core * 128          # nodes per core (padded)
        self.NP = self.NN * ncores              # padded node count
        self.NTILE = self.NP // 128
        assert self.NP >= n_nodes
        self.NCHUNK = nchunk
        assert self.NP % nchunk == 0
        self.CHUNK = self.NP // nchunk          # gather-table rows per chunk
        assert self.CHUNK <= 32767, "int16 gather index range"
        self.G1 = group1                        # layer-1 dst tiles per group
        self.G2 = group2                        # layer-2 dst tiles per group
        self.SUBB = subb                        # layer-1 stream blocks/subcall


REAL_CFG = Cfg(100000, 3200000)


# ----------------------------------------------------------------------------
# Host preprocessing
# ----------------------------------------------------------------------------

def _sched2(cfg, src, dst):
    """Layer-2 gather schedule: (group2, chunk) calls, per-(tile,chunk) segs
    padded to 128, SPMD-uniform across cores. Returns calls, idx, dloc."""
    NP, NN, TPC, NC = cfg.NP, cfg.NN, cfg.TPC, cfg.NC
    G, NCHUNK, CHUNK = cfg.G2, cfg.NCHUNK, cfg.CHUNK
    NGRP = math.ceil(TPC / G)

    core_of = dst // NN
    tile_of = (dst % NN) // 128
    dloc_of = dst % 128
    chunk_of = src // CHUNK

    key = (core_of * TPC + tile_of) * NCHUNK + chunk_of
    counts = np.bincount(key, minlength=NC * TPC * NCHUNK).reshape(
        NC, TPC, NCHUNK)
    seg_len = counts.max(axis=0)
    seg_len = (np.ceil(seg_len / 128).astype(np.int64)) * 128

    order = np.argsort(key, kind="stable")
    src_s, key_s = src[order], key[order]
    dloc_s = dloc_of[order]
    run_starts = np.searchsorted(key_s, np.arange(NC * TPC * NCHUNK))

    TOT = int(seg_len.sum())
    NBLK = TOT // 128
    idx_all = np.zeros((NC, 128, TOT // 16), dtype=np.int16)
    dloc_all = np.full((NC, 128, NBLK), 300.0, dtype=np.float32)

    calls = []
    pos = 0
    for g in range(NGRP):
        t0, t1 = g * G, min((g + 1) * G, TPC)
        for c in range(NCHUNK):
            Lgc = int(seg_len[t0:t1, c].sum())
            if Lgc == 0:
                continue
            blocks = []
            for t in range(t0, t1):
                blocks += [t] * int(seg_len[t, c] // 128)
            SUB = 96
            for sb0 in range(0, len(blocks), SUB):
                sb = blocks[sb0:sb0 + SUB]
                calls.append({"g": g, "c": c, "L": len(sb) * 128,
                              "col16": pos // 16 + sb0 * 8,
                              "blk0": pos // 128 + sb0, "blocks": sb})
            for core in range(NC):
                p = pos
                for t in range(t0, t1):
                    L = int(seg_len[t, c])
                    if L == 0:
                        continue
                    k = (core * TPC + t) * NCHUNK + c
                    s0 = run_starts[k]
                    n = int(counts[core, t, c])
                    seg_idx = np.zeros(L, dtype=np.int16)
                    seg_dl = np.full(L, 300.0, dtype=np.float32)
                    if n:
                        seg_idx[:n] = (src_s[s0:s0 + n] - c * CHUNK).astype(
                            np.int16)
                        seg_dl[:n] = dloc_s[s0:s0 + n]
                    w = seg_idx.reshape(L // 16, 16).T
                    idx_all[core][:, p // 16: p // 16 + L // 16] = \
                        np.tile(w, (8, 1))
                    dloc_all[core][:, p // 128: p // 128 + L // 128] = \
                        seg_dl.reshape(L // 128, 128).T
                    p += L
            pos += Lgc
    assert pos == TOT
    return {"calls": calls, "TOT": TOT, "NBLK": NBLK,
            "maxB": max(c["L"] for c in calls) // 128,
            "idx": idx_all, "dloc": dloc_all}


def _sched1(cfg, src, dst, dinv):
    """Layer-1 stream schedule: chunkless, per-tile segs padded to 128,
    split into subcalls of <= SUBB blocks within each group1.
    Returns calls, per-core (srcs, dloc, dsrc)."""
    NN, TPC, NC, G = cfg.NN, cfg.TPC, cfg.NC, cfg.G1
    NGRP = math.ceil(TPC / G)

    core_of = dst // NN
    tile_of = (dst % NN) // 128
    dloc_of = dst % 128

    key = core_of * TPC + tile_of
    counts = np.bincount(key, minlength=NC * TPC).reshape(NC, TPC)
    seg_len = counts.max(axis=0)
    seg_len = (np.ceil(seg_len / 128).astype(np.int64)) * 128

    order = np.argsort(key, kind="stable")
    src_s, key_s = src[order], key[order]
    dloc_s = dloc_of[order]
    run_starts = np.searchsorted(key_s, np.arange(NC * TPC))

    TOT = int(seg_len.sum())
    NBLK = TOT // 128
    srcs_all = np.zeros((NC, TOT), dtype=np.int64)
    dloc_all = np.full((NC, 128, NBLK), 300.0, dtype=np.float32)
    dsrc_all = np.zeros((NC, 128, NBLK), dtype=np.float32)

    calls = []
    pos = 0
    for g in range(NGRP):
        t0, t1 = g * G, min((g + 1) * G, TPC)
        blocks = []
        for t in range(t0, t1):
            blocks += [t] * int(seg_len[t] // 128)
        for sb0 in range(0, len(blocks), cfg.SUBB):
            sb = blocks[sb0:sb0 + cfg.SUBB]
            calls.append({"g": g, "L": len(sb) * 128,
                          "blk0": pos // 128 + sb0, "blocks": sb})
        for core in range(NC):
            p = pos * 1
            for t in range(t0, t1):
                L = int(seg_len[t])
                k = core * TPC + t
                sidx = run_starts[k]
                n = int(counts[core, t])
                seg_src = np.zeros(L, dtype=np.int64)
                seg_dl = np.full(L, 300.0, dtype=np.float32)
                seg_ds = np.zeros(L, dtype=np.float32)
                if n:
                    seg_src[:n] = src_s[sidx:sidx + n]
                    seg_dl[:n] = dloc_s[sidx:sidx + n]
                    seg_ds[:n] = dinv[seg_src[:n]]
                srcs_all[core][p:p + L] = seg_src
                dloc_all[core][:, p // 128: p // 128 + L // 128] = \
                    seg_dl.reshape(L // 128, 128).T
                dsrc_all[core][:, p // 128: p // 128 + L // 128] = \
                    seg_ds.reshape(L // 128, 128).T
                p += L
        pos += int(seg_len[t0:t1].sum())
    assert pos == TOT
    return {"calls": calls, "TOT": TOT, "NBLK": NBLK,
            "srcs": srcs_all, "dloc": dloc_all, "dsrc": dsrc_all}


def preprocess(cfg, x, edge_index, W1, b1, W2, b2):
    N, NP, NN, TPC, NC = cfg.N, cfg.NP, cfg.NN, cfg.TPC, cfg.NC

    src = np.asarray(edge_index[0], dtype=np.int64)
    dst = np.asarray(edge_index[1], dtype=np.int64)

    deg = np.bincount(dst, minlength=NP).astype(np.float32) + 1.0
    dinv = (1.0 / np.sqrt(deg)).astype(np.float32)

    s1 = _sched1(cfg, src, dst, dinv)
    s2 = _sched2(cfg, src, dst)
    meta = {"s1": s1, "s2": s2}

    xb = np.zeros((NP, cfg.F_IN), dtype=ml_dtypes.bfloat16)
    xb[:N] = np.asarray(x, dtype=np.float32).astype(ml_dtypes.bfloat16)
    dinv_full = np.ascontiguousarray(dinv.reshape(cfg.NTILE, 128).T)
    GW = max(cfg.G1, cfg.G2)
    iota = np.tile(np.tile(np.arange(128, dtype=np.float32), GW)[None, :],
                   (128, 1)).astype(ml_dtypes.bfloat16)
    ident = np.eye(128, dtype=np.float32).astype(ml_dtypes.bfloat16)

    in_maps = []
    for core in range(NC):
        sh = slice(core * NN, (core + 1) * NN)
        # edge-expanded layer-1 stream, slot-major [128, NBLK1*128]
        mx = xb[s1["srcs"][core]]                       # [TOT1, 128] bf16
        mx = np.ascontiguousarray(
            mx.reshape(s1["NBLK"], 128, cfg.F_IN).transpose(1, 0, 2)
            .reshape(128, s1["NBLK"] * cfg.F_IN))
        in_maps.append({
            "mx": mx,
            "dloc1": s1["dloc"][core].astype(ml_dtypes.bfloat16),
            "dsrc1": s1["dsrc"][core].astype(ml_dtypes.bfloat16),
            "idx2": s2["idx"][core],
            "dloc2": s2["dloc"][core].astype(ml_dtypes.bfloat16),
            "xb_own": np.ascontiguousarray(xb[sh]),
            "dinv_own": np.ascontiguousarray(
                dinv_full[:, core * TPC:(core + 1) * TPC]),
            "W1": np.asarray(W1, np.float32).astype(ml_dtypes.bfloat16),
            "W2": np.asarray(W2, np.float32).astype(ml_dtypes.bfloat16),
            "b1": np.tile(np.asarray(b1, np.float32)[None, :], (128, 1)),
            "b2": np.tile(np.asarray(b2, np.float32)[None, :], (128, 1)),
            "iota": iota,
            "ident": ident,
        })
    return in_maps, meta, dinv


# ----------------------------------------------------------------------------
# Device graph
# ----------------------------------------------------------------------------

def build_bass(cfg, meta, debug=False):
    NN, TPC, HID, F_IN, F_OUT = cfg.NN, cfg.TPC, cfg.HID, cfg.F_IN, cfg.F_OUT
    NP, NTILE, NC, CHUNK = cfg.NP, cfg.NTILE, cfg.NC, cfg.CHUNK
    G1, G2 = cfg.G1, cfg.G2
    s1, s2 = meta["s1"], meta["s2"]
    NGRP1 = math.ceil(TPC / G1)
    NGRP2 = math.ceil(TPC / G2)
    MAXB = max(cfg.SUBB, s2["maxB"])
    assert MAXB <= 96, MAXB

    nc = bacc.Bacc("TRN2", target_bir_lowering=False, debug=debug)

    mxp = nc.declare_dram_parameter("mx", [128, s1["NBLK"] * 128], BF16,
                                    isOutput=False)
    dloc1p = nc.declare_dram_parameter("dloc1", [128, s1["NBLK"]], BF16,
                                       isOutput=False)
    dsrc1p = nc.declare_dram_parameter("dsrc1", [128, s1["NBLK"]], BF16,
                                       isOutput=False)
    idx2p = nc.declare_dram_parameter("idx2", [128, s2["TOT"] // 16], I16,
                                      isOutput=False)
    dloc2p = nc.declare_dram_parameter("dloc2", [128, s2["NBLK"]], BF16,
                                       isOutput=False)
    xbop = nc.declare_dram_parameter("xb_own", [NN, F_IN], BF16,
                                     isOutput=False)
    dinvop = nc.declare_dram_parameter("dinv_own", [128, TPC], FP32,
                                       isOutput=False)
    W1p = nc.declare_dram_parameter("W1", [F_IN, HID], BF16, isOutput=False)
    W2p = nc.declare_dram_parameter("W2", [HID, F_OUT], BF16, isOutput=False)
    b1p = nc.declare_dram_parameter("b1", [128, HID], FP32, isOutput=False)
    b2p = nc.declare_dram_parameter("b2", [128, F_OUT], FP32, isOutput=False)
    GW = max(G1, G2)
    iotap = nc.declare_dram_parameter("iota", [128, GW * 128], BF16,
                                      isOutput=False)
    identp = nc.declare_dram_parameter("ident", [128, 128], BF16,
                                       isOutput=False)
    out = nc.declare_dram_parameter("out", [NN, F_OUT], FP32, isOutput=True)

    groups = [list(range(NC))]

    with tile.TileContext(nc) as tc:
        with (
            tc.tile_pool(name="persist", bufs=1) as pp,
            tc.tile_pool(name="dram", bufs=1, space="DRAM") as dp,
            tc.tile_pool(name="xs", bufs=3) as xpool,
            tc.tile_pool(name="idxp", bufs=8) as ipool,
            tc.tile_pool(name="gatp", bufs=3) as gpool,
            tc.tile_pool(name="sel", bufs=4) as spool,
            tc.tile_pool(name="ps_acc", bufs=2, space="PSUM") as ps_acc,
            tc.tile_pool(name="ps_e", bufs=4, space="PSUM") as ps_e,
            tc.tile_pool(name="epi", bufs=4) as epool,
        ):
            # ---- persistent SBUF ----
            W1_s = pp.tile([F_IN, HID], BF16)
            W2_s = pp.tile([HID, F_OUT], BF16)
            b1_s = pp.tile([128, HID], FP32)
            b2_s = pp.tile([128, F_OUT], FP32)
            dinvo_s = pp.tile([128, TPC], FP32)
            iota_s = pp.tile([128, GW * 128], BF16)
            ident_s = pp.tile([128, 128], BF16)
            dloc1_s = pp.tile([128, s1["NBLK"]], BF16)
            dsrc1_s = pp.tile([128, s1["NBLK"]], BF16)
            dloc2_s = pp.tile([128, s2["NBLK"]], BF16)
            xd_own = pp.tile([128, TPC * 128], BF16)
            hs2 = pp.tile([128, TPC * 128], BF16)
            nc.vector.memset(hs2[:], 0.0)
            for t_, d_ in ((W1_s, W1p), (W2_s, W2p), (b1_s, b1p), (b2_s, b2p),
                           (dinvo_s, dinvop), (iota_s, iotap),
                           (ident_s, identp), (dloc1_s, dloc1p),
                           (dsrc1_s, dsrc1p), (dloc2_s, dloc2p)):
                nc.sync.dma_start(out=t_[:], in_=d_[:, :])

            # ---- DRAM temps ----
            shard2 = dp.tile([NN, 128], BF16)
            table2 = dp.tile([NP, 128], BF16)

            # ---- own-shard x*dinv tiles in SBUF (self-loop terms) ----
            XB = 8
            for b0 in range(0, TPC, XB):
                nb_ = min(XB, TPC - b0)
                xt = xpool.tile([128, XB * 128], BF16, tag="xt")
                nc.sync.dma_start(
                    out=xt[:, :nb_ * 128].rearrange("p (t f) -> p t f", f=F_IN),
                    in_=xbop[b0 * 128:(b0 + nb_) * 128, :].rearrange(
                        "(t p) f -> p t f", p=128))
                nc.vector.tensor_tensor(
                    out=xd_own[:, b0 * 128:(b0 + nb_) * 128].rearrange(
                        "p (t f) -> p t f", f=F_IN),
                    in0=xt[:, :nb_ * 128].rearrange("p (t f) -> p t f", f=F_IN),
                    in1=dinvo_s[:, b0:b0 + nb_].to_broadcast([128, nb_, F_IN]),
                    op=AL.mult)

            def self_matmuls(pacc, t0, TG, F, own_sb):
                for tt in range(TG):
                    col0 = tt * F
                    first = (col0 % 512) == 0
                    nc.tensor.matmul(
                        out=pacc[:, col0:col0 + F],
                        lhsT=ident_s[:],
                        rhs=own_sb[:, (t0 + tt) * 128:(t0 + tt) * 128 + F],
                        start=first, stop=first,
                        skip_group_check=True)

            def sel_build(dloc_sb, blk0, j, w):
                selw = spool.tile([128, GW * 128], BF16, tag="selw",
                                  name="selw")
                nc.vector.tensor_tensor(
                    out=selw[:, :w * 128].rearrange("p (b m) -> p b m", m=128),
                    in0=iota_s[:, :w * 128].rearrange("p (b m) -> p b m",
                                                      m=128),
                    in1=dloc_sb[:, blk0 + j:blk0 + j + w
                                ].to_broadcast([128, w, 128]),
                    op=AL.is_equal)
                return selw

            # ================= layer 1: streamed aggregation =================
            def layer1():
                G = G1
                for g in range(NGRP1):
                    t0 = g * G
                    TG = min(G, TPC - t0)
                    pacc = ps_acc.tile([128, G1 * 128], FP32, space="PSUM",
                                       tag="pacc", name="pacc")
                    self_matmuls(pacc, t0, TG, 128, xd_own)
                    for call in s1["calls"]:
                        if call["g"] != g:
                            continue
                        nb, blk0 = call["L"] // 128, call["blk0"]
                        gat = gpool.tile([128, MAXB * 128], BF16, tag="gat",
                                         name="gat")
                        nc.sync.dma_start(
                            out=gat[:, :nb * 128],
                            in_=mxp[:, blk0 * 128:(blk0 + nb) * 128])
                        # scale by dinv[src]
                        nc.vector.tensor_tensor(
                            out=gat[:, :nb * 128].rearrange(
                                "p (b f) -> p b f", f=128),
                            in0=gat[:, :nb * 128].rearrange(
                                "p (b f) -> p b f", f=128),
                            in1=dsrc1_s[:, blk0:blk0 + nb
                                        ].to_broadcast([128, nb, 128]),
                            op=AL.mult)
                        selw = None
                        for j, t in enumerate(call["blocks"]):
                            if j % G == 0:
                                w = min(G, nb - j)
                                selw = sel_build(dloc1_s, blk0, j, w)
                            nc.tensor.matmul(
                                out=pacc[:, (t - t0) * 128:(t - t0 + 1) * 128],
                                lhsT=selw[:, (j % G) * 128:(j % G + 1) * 128],
                                rhs=gat[:, j * 128:(j + 1) * 128],
                                start=False, stop=False,
                                skip_group_check=True)
                    for tt in range(TG):
                        t = t0 + tt
                        v = epool.tile([128, 128], BF16, tag="v", name="v")
                        nc.scalar.activation(
                            out=v[:], in_=pacc[:, tt * 128:(tt + 1) * 128],
                            func=AF.Copy, scale=dinvo_s[:, t:t + 1])
                        pt = ps_e.tile([128, 128], BF16, space="PSUM",
                                       tag="t", bufs=2, name="pt")
                        nc.tensor.transpose(out=pt[:], in_=v[:],
                                            identity=ident_s[:])
                        vT = epool.tile([128, 128], BF16, tag="vT", name="vT")
                        nc.scalar.activation(out=vT[:], in_=pt[:],
                                             func=AF.Copy)
                        ph = ps_e.tile([128, 128], FP32, space="PSUM", tag="e",
                                       bufs=2, name="ph")
                        nc.tensor.matmul(out=ph[:, :HID], lhsT=vT[:],
                                         rhs=W1_s[:], start=True, stop=True)
                        hh = epool.tile([128, HID], FP32, tag="hh", name="hh")
                        nc.vector.tensor_tensor(out=hh[:], in0=ph[:, :HID],
                                                in1=b1_s[:], op=AL.add)
                        nc.scalar.activation(
                            out=hs2[:, t * 128:t * 128 + HID], in_=hh[:],
                            func=AF.Relu, scale=dinvo_s[:, t:t + 1])

            # ================= layer 2: gathered aggregation =================
            def layer2():
                G = G2
                for g in range(NGRP2):
                    t0 = g * G
                    TG = min(G, TPC - t0)
                    pacc = ps_acc.tile([128, G2 * HID], FP32, space="PSUM",
                                       tag="pacc", name="pacc")
                    self_matmuls(pacc, t0, TG, HID, hs2)
                    for call in s2["calls"]:
                        if call["g"] != g:
                            continue
                        c, L = call["c"], call["L"]
                        nb, blk0 = L // 128, call["blk0"]
                        idxt = ipool.tile([128, (MAXB * 128) // 16], I16,
                                          tag="idx", name="idxt")
                        nc.sync.dma_start(
                            out=idxt[:, :L // 16],
                            in_=idx2p[:, call["col16"]:call["col16"] + L // 16])
                        gat = gpool.tile([128, MAXB * 128], BF16, tag="gat",
                                         name="gat")
                        nc.gpsimd.dma_gather(
                            out_ap=gat[:, :nb * 128].rearrange(
                                "p (b f) -> p b f", f=128),
                            in_ap=table2[c * CHUNK:(c + 1) * CHUNK, :],
                            idxs_ap=idxt[:, :L // 16],
                            num_idxs=L, num_idxs_reg=L, elem_size=128,
                            single_packet=False)
                        selw = None
                        for j, t in enumerate(call["blocks"]):
                            if j % G == 0:
                                w = min(G, nb - j)
                                selw = sel_build(dloc2_s, blk0, j, w)
                            nc.tensor.matmul(
                                out=pacc[:, (t - t0) * HID:(t - t0 + 1) * HID],
                                lhsT=selw[:, (j % G) * 128:(j % G + 1) * 128],
                                rhs=gat[:, j * 128:j * 128 + HID],
                                start=False, stop=False,
                                skip_group_check=True)
                    for tt in range(TG):
                        t = t0 + tt
                        u2 = epool.tile([128, HID], BF16, tag="v", name="u2")
                        nc.scalar.activation(
                            out=u2[:], in_=pacc[:, tt * HID:(tt + 1) * HID],
                            func=AF.Copy, scale=dinvo_s[:, t:t + 1])
                        pt = ps_e.tile([128, 128], BF16, space="PSUM",
                                       tag="t", bufs=2, name="pt")
                        nc.tensor.transpose(out=pt[:HID, :], in_=u2[:],
                                            identity=ident_s[:])
                        u2T = epool.tile([HID, 128], BF16, tag="vT", name="u2T")
                        nc.scalar.activation(out=u2T[:], in_=pt[:HID, :],
                                             func=AF.Copy)
                        po = ps_e.tile([128, 128], FP32, space="PSUM", tag="e",
                                       bufs=2, name="po")
                        nc.tensor.matmul(out=po[:, :F_OUT], lhsT=u2T[:],
                                         rhs=W2_s[:], start=True, stop=True)
                        oo = epool.tile([128, F_OUT], FP32, tag="oo", name="oo")
                        nc.vector.tensor_tensor(out=oo[:], in0=po[:, :F_OUT],
                                                in1=b2_s[:], op=AL.add)
                        nc.sync.dma_start(out=out[t * 128:(t + 1) * 128, :],
                                          in_=oo[:])

            layer1()
            nc.sync.dma_start(
                out=shard2[:].rearrange("(t p) f -> p t f", p=128),
                in_=hs2[:].rearrange("p (t f) -> p t f", f=128))
            nc.gpsimd.collective_compute(
                "AllGather", AL.bypass, replica_groups=groups,
                ins=[shard2[:].opt()], outs=[table2[:].opt()])
            layer2()

    return nc


def hoist_gather_waits(nc):
    """walrus's ANT codegen dies ("Reg has not been allocated yet") when a
    DMAGatherAnt carries an attached semaphore wait. Move any waits Tile
    attached onto a fresh no-op right before the gather."""
    gather_ops = (mybir.InstDMAGatherAnt, mybir.InstDMAScatterAddAnt)
    for blk in nc.main_func.blocks:
        insts = blk.instructions
        i = 0
        while i < len(insts):
            ins = insts[i]
            if isinstance(ins, gather_ops) and ins.sync_info is not None \
                    and len(ins.sync_info.on_wait) > 0:
                nop = mybir.InstNoOp(
                    name=f"gw-nop-{ins.name}",
                    ins=[], outs=[],
                    engine=ins.engine,
                    sync_info=mybir.SyncInfo(
                        on_wait=list(ins.sync_info.on_wait), on_update=[]),
                    text_hint="hoisted-gather-waits",
                    bass_nofuse=True,
                )
                ins.sync_info.on_wait = []
                insts.insert(i, nop)
                i += 1
            i += 1


# ----------------------------------------------------------------------------
# Entry points
# ----------------------------------------------------------------------------

def run_on_hw(cfg, in_maps, meta, trace=False, tmpdir=None):
    from concourse.bass_utils import run_bass_kernel_spmd
    nc = build_bass(cfg, meta, debug=False)
    hoist_gather_waits(nc)
    nc.finalize()
    res = run_bass_kernel_spmd(nc, in_maps, core_ids=list(range(cfg.NC)),
                               trace=trace, tmpdir=tmpdir)
    outs = [res.results[c]["out"] for c in range(cfg.NC)]
    full = np.concatenate(outs, axis=0)[:cfg.N]
    return full, res


def kernel(x, edge_index, W1, b1, W2, b2):
    cfg = REAL_CFG
    in_maps, meta, _ = preprocess(cfg, x, edge_index, W1, b1, W2, b2)
    out, _ = run_on_hw(cfg, in_maps, meta, trace=False)
    return out.astype(np.float32)
